# revision 1
# baseline (speedup 1.0000x reference)
"""Bass/Trainium2 kernel for a single-head causal decoder attention head.

Reference computation (fp32):
    k = x @ Wk; q = x @ Wq; v = x @ Wv            # [B,T,H]
    att = softmax(causal(q k^T / sqrt(H)))        # [B,T,T]
    out = att @ v                                 # [B,T,H]
with B=4, T=4096, C=1024, H=128.

Sharding: 8 cores = 4 batches x 2 query-interleave lanes (j in {0,1}).
Core (b, j) handles q-groups [(2i+j)*512, (2i+j+1)*512) for i in 0..3 and
runs a *uniform* kv-span schedule {1024, 2048, 3072, 4096} for groups
0..3, so all 8 cores execute the same instruction stream (SPMD, one NEFF)
while per-core DRAM data (x^T slices, q-column gather, mask stack) makes
the math come out right.  Causality beyond each group's true span is
enforced by additive -30000 masks on the last 8 kv chunks of each group.

Dataflow (per core, transposed land so no on-chip transposes are needed):
    KT [H, T]   = Wk^T x^T        (8 c-chunk matmuls per 512 kv cols)
    QT [H, 2048]= Wq^T xq^T
    V  [kv, H]  (32 blocks)       (lhsT = x^T chunk, rhs = Wv chunk)
    per q-group, per kv chunk c:
        S^T  = KT_c^T QT_g                 (PSUM [128kv, 512q])
        S^T += mask (last 8 chunks, DVE)
        P^T  = exp(S^T / sqrt(H))          (ACT, bf16 -> SBUF)
        outT += V_c^T P^T                  (PSUM [128H, 512q], accumulated)
        sums += ones^T P^T                 (PSUM [1, 512q], accumulated)
    outT / sums -> DRAM  (reciprocal + partition-broadcast + multiply)
"""

import sys

sys.path.insert(0, "/opt/trn_rl_repo")

import numpy as np
import ml_dtypes

import concourse.bass as bass
import concourse.mybir as mybir
import concourse.tile as tile
from concourse import bacc
from concourse.alu_op_type import AluOpType
from concourse.masks import make_identity
from concourse.bass_utils import run_bass_kernel_spmd

B, T, C, H = 4, 4096, 1024, 128
NCORES = 8
QG = 512                      # q-group width
NG = 4                        # q-groups per core
SPANS = [1024, 2048, 3072, 4096]  # uniform kv span per group index
CB = C // 128                 # 8 contraction chunks
TGRP = T // QG                # 8 kv col-groups for projections
SCALE = float(H) ** -0.5
MASKVAL = -30000.0

BF16 = mybir.dt.bfloat16
F32 = mybir.dt.float32
NPBF16 = ml_dtypes.bfloat16


def _build_program():
    nc = bacc.Bacc("TRN2", target_bir_lowering=False, debug=False)

    xt = nc.dram_tensor("xt", [C, T], BF16, kind="ExternalInput").ap()
    xtq = nc.dram_tensor("xtq", [C, NG * QG], BF16, kind="ExternalInput").ap()
    wk = nc.dram_tensor("wk", [C, H], BF16, kind="ExternalInput").ap()
    wq = nc.dram_tensor("wq", [C, H], BF16, kind="ExternalInput").ap()
    wv = nc.dram_tensor("wv", [C, H], BF16, kind="ExternalInput").ap()
    msk = nc.dram_tensor("msk", [8, 128, QG], BF16, kind="ExternalInput").ap()
    outT = nc.dram_tensor("outT", [H, NG * QG], F32, kind="ExternalOutput").ap()

    with tile.TileContext(nc) as tc:
        with (
            tc.tile_pool(name="const", bufs=1) as constp,
            tc.tile_pool(name="kvq", bufs=1) as kvqp,
            tc.tile_pool(name="xin", bufs=2) as xinp,
            tc.tile_pool(name="attb", bufs=4) as attp,
            tc.tile_pool(name="epi", bufs=2) as epip,
        ):
            # --- persistent SBUF tensors ---
            wks = constp.tile([128, CB * H], BF16, tag="wks")
            wqs = constp.tile([128, CB * H], BF16, tag="wqs")
            wvs = constp.tile([128, CB * H], BF16, tag="wvs")
            for eng, ws, w in (
                (nc.scalar, wks, wk), (nc.scalar, wqs, wq), (nc.gpsimd, wvs, wv)
            ):
                eng.dma_start(
                    ws.rearrange("p (c h) -> p c h", c=CB),
                    w.rearrange("(c p) h -> p c h", p=128),
                )
            masks = constp.tile([128, 8 * QG], BF16, tag="masks")
            ident = constp.tile([128, 128], BF16, tag="ident")
            make_identity(nc, ident)

            KT = kvqp.tile([128, T], BF16, tag="KT")
            VT = kvqp.tile([128, T], BF16, tag="VT")
            VV = kvqp.tile([128, (T // 128) * H], BF16, tag="VV")
            QT = kvqp.tile([128, NG * QG], BF16, tag="QT")
            ones = kvqp.tile([128, 128], BF16, tag="ones")
            nc.vector.memset(ones, 1.0)

            # --- phase 1: projections ---
            with tc.tile_pool(name="pp", bufs=2, space="PSUM") as ppool:
                xtr = xt.rearrange("(c p) t -> p c t", p=128)
                for tg in range(TGRP):
                    xg = xinp.tile([128, CB * QG], BF16, tag="xg", bufs=TGRP)
                    xgv = xg.rearrange("p (c q) -> p c q", c=CB)
                    if tg == 0:
                        nc.sync.dma_start(xgv[:, 0:1], xtr[:, 0:1, 0:QG])
                        nc.sync.dma_start(xgv[:, 1:CB], xtr[:, 1:CB, 0:QG])
                    else:
                        nc.sync.dma_start(
                            xgv, xtr[:, :, tg * QG:(tg + 1) * QG]
                        )
                    kps = ppool.tile([128, QG], F32, tag="kps")
                    for c in range(CB):
                        nc.tensor.matmul(
                            kps,
                            lhsT=wks[:, c * H:(c + 1) * H],
                            rhs=xg[:, c * QG:(c + 1) * QG],
                            start=(c == 0),
                            stop=(c == CB - 1),
                        )
                    nc.any.tensor_copy(KT[:, tg * QG:(tg + 1) * QG], kps)
                    vps = ppool.tile([128, QG], F32, tag="vps")
                    for c in range(CB):
                        nc.tensor.matmul(
                            vps,
                            lhsT=wvs[:, c * H:(c + 1) * H],
                            rhs=xg[:, c * QG:(c + 1) * QG],
                            start=(c == 0),
                            stop=(c == CB - 1),
                        )
                    nc.any.tensor_copy(VT[:, tg * QG:(tg + 1) * QG], vps)
                    for tb in range(QG // 128):
                        t = tg * (QG // 128) + tb
                        tps = ppool.tile([128, 128], BF16, tag="tps")
                        nc.tensor.transpose(
                            tps, VT[:, t * 128:(t + 1) * 128], ident
                        )
                        nc.vector.tensor_copy(VV[:, t * H:(t + 1) * H], tps)
                for i in range(NG):
                    xq = xinp.tile([128, CB * QG], BF16, tag="xq", bufs=NG)
                    nc.sync.dma_start(
                        xq.rearrange("p (c q) -> p c q", c=CB),
                        xtq.rearrange("(c p) t -> p c t", p=128)[:, :, i * QG:(i + 1) * QG],
                    )
                    qps = ppool.tile([128, QG], F32, tag="qps")
                    for c in range(CB):
                        nc.tensor.matmul(
                            qps,
                            lhsT=wqs[:, c * H:(c + 1) * H],
                            rhs=xq[:, c * QG:(c + 1) * QG],
                            start=(c == 0),
                            stop=(c == CB - 1),
                        )
                    nc.any.tensor_copy(QT[:, i * QG:(i + 1) * QG], qps)

            # --- phase 2: attention ---
            nc.sync.dma_start(
                masks.rearrange("p (m q) -> p m q", m=8),
                msk.rearrange("m p q -> p m q"),
            )
            with tc.tile_pool(name="ap", bufs=2, space="PSUM") as apool:
                for i in range(NG):
                    span = SPANS[i]
                    nchunks = span // 128
                    otps = apool.tile([128, QG], F32, tag="otps", bufs=1)
                    smps = apool.tile([128, QG], F32, tag="smps", bufs=1)
                    qg = QT[:, i * QG:(i + 1) * QG]
                    for cp in range(nchunks // 2):
                        c0 = 2 * cp
                        sps = apool.tile([128, 2 * QG], F32, tag="sps", bufs=3)
                        for h in range(2):
                            nc.tensor.matmul(
                                sps[:, h * QG:(h + 1) * QG],
                                lhsT=KT[:, (c0 + h) * 128:(c0 + h + 1) * 128],
                                rhs=qg,
                                start=True,
                                stop=True,
                            )
                        pt = attp.tile([128, 2 * QG], BF16, tag="pt")
                        nc.scalar.activation(
                            pt, sps, mybir.ActivationFunctionType.Exp, scale=SCALE
                        )
                        m = c0 - (nchunks - 8)
                        if m >= 0:
                            nc.vector.tensor_tensor(
                                pt, pt, masks[:, m * QG:(m + 2) * QG],
                                op=AluOpType.mult,
                            )
                        for h in range(2):
                            c = c0 + h
                            ph = pt[:, h * QG:(h + 1) * QG]
                            nc.tensor.matmul(
                                otps,
                                lhsT=VV[:, c * H:(c + 1) * H],
                                rhs=ph,
                                start=(c == 0),
                                stop=(c == nchunks - 1),
                            )
                            nc.tensor.matmul(
                                smps,
                                lhsT=ones,
                                rhs=ph,
                                start=(c == 0),
                                stop=(c == nchunks - 1),
                            )
                    rb = epip.tile([128, QG], F32, tag="rb")
                    nc.vector.reciprocal_approx_fast(rb, smps)
                    ot = epip.tile([128, QG], F32, tag="ot")
                    nc.vector.tensor_tensor(ot, otps, rb, op=AluOpType.mult)
                    nc.sync.dma_start(outT[:, i * QG:(i + 1) * QG], ot)

    if not nc.is_finalized():
        nc.finalize()
    return nc


_NC_CACHE = None


def _get_program():
    global _NC_CACHE
    if _NC_CACHE is None:
        _NC_CACHE = _build_program()
    return _NC_CACHE


def _make_masks(j: int) -> np.ndarray:
    """Mask stack [8, 128, QG] for lane j (f32, 0 or MASKVAL).

    Slot s applies to kv chunk at offset K0 = g - (1024 - j*512) + 128*s
    relative ... concretely: for lane j, the last 8 chunks of each group's
    span get slots 0..7; masked iff global kv > global q, i.e.
    128*(s - 4 + (1 - j) * 4 ... reduces to: kv_i + 128*s - (4 - 4*j)*128 > q_j
    """
    out = np.zeros((8, 128, QG), NPBF16)
    kv = np.arange(128)[:, None]
    q = np.arange(QG)[None, :]
    for s in range(8):
        # lane j: slot s covers the chunk at K0 = g + 128*s - 512*j;
        # multiplicative mask: 0 where kv_global > q_global else 1
        rel = 128 * s - 512 * j
        out[s] = np.where(rel + kv > q, 0.0, 1.0).astype(NPBF16)
    return out


def _run(inputs: dict, trace: bool = False, trace_kwargs: dict | None = None):
    x = np.asarray(inputs["x"], np.float32)
    Wk = np.asarray(inputs["Wk"], np.float32)
    Wq = np.asarray(inputs["Wq"], np.float32)
    Wv = np.asarray(inputs["Wv"], np.float32)

    nc = _get_program()

    wk16 = Wk.astype(NPBF16)
    wq16 = Wq.astype(NPBF16)
    wv16 = Wv.astype(NPBF16)
    msks = [_make_masks(j) for j in range(2)]

    in_maps = []
    for b in range(B):
        xtb = np.ascontiguousarray(x[b].T).astype(NPBF16)  # [C, T]
        for j in range(2):
            xtq = np.concatenate(
                [xtb[:, (2 * i + j) * QG:(2 * i + j + 1) * QG] for i in range(NG)],
                axis=1,
            )
            in_maps.append(
                {
                    "xt": xtb,
                    "xtq": np.ascontiguousarray(xtq),
                    "wk": wk16,
                    "wq": wq16,
                    "wv": wv16,
                    "msk": msks[j],
                }
            )

    res = run_bass_kernel_spmd(
        nc,
        in_maps,
        core_ids=list(range(NCORES)),
        trace=trace,
        **(trace_kwargs or {}),
    )

    out = np.empty((B, T, H), np.float32)
    for core in range(NCORES):
        b, j = divmod(core, 2)
        oT = np.asarray(res.results[core]["outT"], np.float32)  # [H, NG*QG]
        for i in range(NG):
            g = (2 * i + j) * QG
            out[b, g:g + QG, :] = oT[:, i * QG:(i + 1) * QG].T
    return out, res


def kernel(**inputs) -> np.ndarray:
    out, _ = _run(inputs, trace=False)
    return out



# revision 22
# speedup vs baseline: 1.2797x; 1.2797x over previous
"""Bass/Trainium2 kernel for a single-head causal decoder attention head.

Reference (fp32):
    k = x @ Wk; q = x @ Wq; v = x @ Wv            # [B,T,H]
    att = softmax(causal(q k^T / sqrt(H)))        # [B,T,T]
    out = att @ v                                 # [B,T,H]
with B=4, T=4096, C=1024, H=128.

Sharding: 8 cores = 4 batches x 2 query-interleave lanes (j in {0,1}).
The host permutes kv 512-column blocks per lane (lane 1 swaps pairs:
[1,0,3,2,5,4,7,6]) so ONE SPMD program serves both lanes: q-groups sit
at even slots, per-group kv spans are slot-prefixes {1024,...,4096},
triangle masks are lane-independent (mask slots 0-3 of each group), and
the trailing 4 constant-mask chunks are handled by a per-lane additive
exp bias (-32 kills them on lane 0, -2 keeps them on lane 1).

Precision (tolerance 2e-2 absmax/max|out|; measured ~1.2e-2):
  - Slots 0-1 project in bf16, slots 2-7 in fp8e4 DoubleRow with real
    contraction pairs (2 c-chunks per pass = 2x PE throughput).  Rows
    0..1023 therefore see only bf16-projected K/V/Q on their live kv;
    every fp8-noised path averages over >= 513 kv positions.
  - S matmuls are bf16 (fp8 gives no speedup there: the PE is
    output-column limited; DoublePixel/DoubleColumn measured no faster).
  - exp runs on ACT in [128,1024] chunk-pairs with bias -2 so P fits
    fp8e4 (max 240); the -2 cancels in out = (P V)/(sum P).
  - Non-triangle pairs of groups 1-3 and all const pairs store P in fp8
    and run PV + row-sums as fp8-DR chunk-pair matmuls (2x).  Triangle
    pairs + group-0 plain pairs stay bf16: DVE mask-multiply, PV bf16,
    row-sums via DVE bf16 running adds merged by one ones-matmul.

Schedule: single TileContext; software pipeline where PV/sums of pair
p-1 are emitted after S/exp of pair p, each group's epilogue is emitted
inside the next group's first pair, and projection work for the next
slot-pair is drip-fed between pairs, so the in-order PE queue never
blocks on ACT results.  PSUM: one shared "sps" ring [128,1024]x3 (also
used by projection/transpose psums) + otps + smps = exactly 8 banks.
"""

import os
import sys

sys.path.insert(0, "/opt/trn_rl_repo")

import numpy as np
import ml_dtypes

import concourse.bass as bass
import concourse.mybir as mybir
import concourse.tile as tile
from concourse import bacc
from concourse.alu_op_type import AluOpType
from concourse.masks import make_identity
from concourse.bass_utils import run_bass_kernel_spmd

B, T, C, H = 4, 4096, 1024, 128
NCORES = 8
QG = 512
NG = 4                        # q-groups per core
CB = C // 128                 # 8 contraction chunks
TGRP = 8                      # kv 512-col slots
SCALE = float(H) ** -0.5

S_FP8 = False                 # S matmul stays bf16 (DoublePixel measured no faster)
PROJ8 = os.environ.get("PROJ8", "1") == "1"   # fp8-DR projections for slots>=NBF
NBF = 2                       # first NBF slots project in bf16
ATT8 = os.environ.get("ATT8", "1") == "1"     # fp8 PV/sums pairs in groups 1-3

BF16 = mybir.dt.bfloat16
F32 = mybir.dt.float32
FP8 = mybir.dt.float8e4
NPBF16 = ml_dtypes.bfloat16
NPF8 = ml_dtypes.float8_e4m3
DR = mybir.MatmulPerfMode.DoubleRow
DP = mybir.MatmulPerfMode.DoublePixel


def _build_program():
    nc = bacc.Bacc("TRN2", target_bir_lowering=False, debug=False)

    xbf = nc.dram_tensor("xbf", [C, TGRP * QG], BF16, kind="ExternalInput").ap()
    x8 = nc.dram_tensor("x8", [128, 6, 4, 2, QG], FP8, kind="ExternalInput").ap()
    wk = nc.dram_tensor("wk", [C, H], BF16, kind="ExternalInput").ap()
    wq = nc.dram_tensor("wq", [C, H], BF16, kind="ExternalInput").ap()
    wv = nc.dram_tensor("wv", [C, H], BF16, kind="ExternalInput").ap()
    wk8 = nc.dram_tensor("wk8", [128, 4, 2, H], FP8, kind="ExternalInput").ap()
    wq8 = nc.dram_tensor("wq8", [128, 4, 2, H], FP8, kind="ExternalInput").ap()
    wv8 = nc.dram_tensor("wv8", [128, 4, 2, H], FP8, kind="ExternalInput").ap()
    msk = nc.dram_tensor("msk", [4, 128, QG], BF16, kind="ExternalInput").ap()
    bia = nc.dram_tensor("bia", [128, 4], F32, kind="ExternalInput").ap()
    outT = nc.dram_tensor("outT", [H, NG * QG], F32, kind="ExternalOutput").ap()

    with tile.TileContext(nc) as tc:
        with (
            tc.tile_pool(name="const", bufs=1) as constp,
            tc.tile_pool(name="kvq", bufs=1) as kvqp,
            tc.tile_pool(name="work", bufs=2) as workp,
            tc.tile_pool(name="at", bufs=1, space="PSUM") as atp,
        ):
            # ---------- persistent SBUF ----------
            # DMA issue order is tuned so the first projections can start
            # as early as possible: sync queue carries wks, xb slot0, wqs,
            # xb slot1, then the fp8 x slices; gpsimd carries wvs + fp8
            # weights; the scalar queue carries masks + biases.
            wks = constp.tile([128, CB * H], BF16, tag="wks")
            wqs = constp.tile([128, CB * H], BF16, tag="wqs")
            wvs = constp.tile([128, CB * H], BF16, tag="wvs")
            wk8s = constp.tile([128, 4, 2, H], FP8, tag="wk8s")
            wq8s = constp.tile([128, 4, 2, H], FP8, tag="wq8s")
            wv8s = constp.tile([128, 4, 2, H], FP8, tag="wv8s")
            masks = constp.tile([128, 4 * QG], BF16, tag="masks")
            biasT = constp.tile([128, 4], F32, tag="biasT")
            ident = constp.tile([128, 128], BF16, tag="ident")
            ones = constp.tile([128, 128], BF16, tag="ones")
            ones8 = constp.tile([128, 2, 128], FP8, tag="ones8")

            if S_FP8:
                KTb = kvqp.tile([128, 2 * QG], BF16, tag="KTb")
                KT8 = kvqp.tile([128, T], FP8, tag="KT8")
                QTb = kvqp.tile([128, QG], BF16, tag="QTb")
                QT8 = kvqp.tile([128, 3 * QG], FP8, tag="QT8")
            else:
                KTb = kvqp.tile([128, T], BF16, tag="KTb")
                QTb = kvqp.tile([128, NG * QG], BF16, tag="QTb")
            VT = kvqp.tile([128, T], BF16, tag="VT")
            VV = kvqp.tile([128, T], BF16, tag="VV")    # [p, (32 blk) h]
            VV8 = kvqp.tile([128, T], FP8, tag="VV8")
            VVr = VV.rearrange("p (c h) -> p c h", c=T // 128)
            VV8r = VV8.rearrange("p (c h) -> p c h", c=T // 128)

            nbf = NBF if PROJ8 else TGRP
            xb = [
                kvqp.tile([128, CB, QG], BF16, tag=f"xb{s}", name=f"xb{s}")
                for s in range(nbf)
            ]
            x8s = [
                kvqp.tile([128, 4, 2, QG], FP8, tag=f"x8s{s}", name=f"x8s{s}")
                for s in range(6)
            ]
            xbr = xbf.rearrange("(c p) q -> p c q", p=128)

            def _xb_dma(sl):
                half = CB // 2
                for hh in range(2):
                    nc.sync.dma_start(
                        xb[sl][:, hh * half:(hh + 1) * half],
                        xbr[:, hh * half:(hh + 1) * half,
                            sl * QG:(sl + 1) * QG],
                    )

            def _w_dma(eng, ws, w):
                wr = ws.rearrange("p (c h) -> p c h", c=CB)
                src = w.rearrange("(c p) h -> p c h", p=128)
                half = CB // 2
                for hh in range(2):
                    eng.dma_start(
                        wr[:, hh * half:(hh + 1) * half],
                        src[:, hh * half:(hh + 1) * half],
                    )

            # PE warmup: ident is computed on-chip (no DMA), so these
            # matmuls run during the ~16us DMA bring-up window and ramp the
            # tensor engine to full p-state before the real work arrives.
            make_identity(nc, ident)
            nc.vector.memset(ones, 1.0)
            nc.vector.memset(ones8, 1.0)
            for wu in range(5):
                wps = atp.tile([128, 2 * QG], F32, tag="sps", bufs=3,
                               name="wps")
                for r in range(8):
                    nc.tensor.matmul(
                        wps[:, 0:128], lhsT=ident, rhs=ident,
                        start=(r == 0), stop=(r == 7),
                    )

            _w_dma(nc.sync, wks, wk)
            _xb_dma(0)
            _w_dma(nc.sync, wqs, wq)
            for sl in range(1, nbf):
                _xb_dma(sl)
            _w_dma(nc.gpsimd, wvs, wv)
            for ws, w in ((wk8s, wk8), (wq8s, wq8), (wv8s, wv8)):
                nc.gpsimd.dma_start(ws, w)
            nc.scalar.dma_start(
                masks.rearrange("p (m q) -> p m q", m=4),
                msk.rearrange("m p q -> p m q"),
            )
            nc.scalar.dma_start(biasT, bia)
            if PROJ8:
                for sl in range(6):
                    nc.sync.dma_start(x8s[sl], x8[:, sl])

            VV_SLOTS = {0, 2, 4, 6}        # blocks used by bf16 (tri) PV pairs
            VV8_SLOTS = {0, 1, 2, 3, 4, 5, 7}  # blocks used by fp8 PV pairs

            # ---------- projection emitters ----------
            def proj_bf16(dst_ps, wsrc, s):
                for c in range(CB):
                    nc.tensor.matmul(
                        dst_ps,
                        lhsT=wsrc[:, c * H:(c + 1) * H],
                        rhs=xb[s][:, c],
                        start=(c == 0),
                        stop=(c == CB - 1),
                    )

            def proj_fp8(dst_ps, w8src, s):
                assert s >= 2, "fp8 x tiles only exist for slots 2-7"
                for cp in range(4):
                    nc.tensor.matmul(
                        dst_ps,
                        lhsT=w8src[:, cp],
                        rhs=x8s[s - 2][:, cp],
                        start=(cp == 0),
                        stop=(cp == 3),
                        perf_mode=DR,
                    )

            def emit_K(s):
                kps = atp.tile([128, 2 * QG], F32, tag="sps", bufs=3,
                               name="kps")[:, 0:QG]
                if not PROJ8 or s < NBF:
                    proj_bf16(kps, wks, s)
                else:
                    proj_fp8(kps, wk8s, s)
                if not S_FP8 or s < 2:
                    nc.vector.tensor_copy(KTb[:, s * QG:(s + 1) * QG], kps)
                if S_FP8:
                    nc.vector.tensor_copy(KT8[:, s * QG:(s + 1) * QG], kps)

            def emit_V(s):
                vps = atp.tile([128, 2 * QG], F32, tag="sps", bufs=3,
                               name="vps")[:, 0:QG]
                if not PROJ8 or s < NBF:
                    proj_bf16(vps, wvs, s)
                else:
                    proj_fp8(vps, wv8s, s)
                nc.vector.tensor_copy(VT[:, s * QG:(s + 1) * QG], vps)

            def emit_T(s):
                tps = atp.tile([128, 2 * QG], BF16, tag="sps", bufs=3,
                               name="tps")[:, 0:QG]
                for tb in range(4):
                    nc.tensor.transpose(
                        tps[:, tb * 128:(tb + 1) * 128],
                        VT[:, s * QG + tb * 128:s * QG + (tb + 1) * 128],
                        ident,
                    )
                if not ATT8 or s in VV_SLOTS:
                    nc.vector.tensor_copy(VV[:, s * QG:(s + 1) * QG], tps)
                if ATT8 and s in VV8_SLOTS:
                    nc.vector.tensor_copy(VV8[:, s * QG:(s + 1) * QG], tps)

            def emit_Q(s):
                g = s // 2
                qps = atp.tile([128, 2 * QG], F32, tag="sps", bufs=3,
                               name="qps")[:, 0:QG]
                if not PROJ8 or s < NBF:
                    proj_bf16(qps, wqs, s)
                else:
                    proj_fp8(qps, wq8s, s)
                if not S_FP8:
                    nc.vector.tensor_copy(QTb[:, g * QG:(g + 1) * QG], qps)
                elif g == 0:
                    nc.vector.tensor_copy(QTb, qps)
                else:
                    nc.vector.tensor_copy(
                        QT8[:, (g - 1) * QG:g * QG], qps
                    )

            def emit_K0_half(hcol):
                kps = atp.tile([128, 2 * QG], F32, tag="sps", bufs=3,
                               name="kps")[:, 0:QG // 2]
                for c in range(CB):
                    nc.tensor.matmul(
                        kps,
                        lhsT=wks[:, c * H:(c + 1) * H],
                        rhs=xb[0][:, c, hcol * 256:(hcol + 1) * 256],
                        start=(c == 0),
                        stop=(c == CB - 1),
                    )
                nc.vector.tensor_copy(
                    KTb[:, hcol * 256:(hcol + 1) * 256], kps)

            # head: only K0+Q0 before attention; everything else is fed
            # through the per-pair pending queue (group g needs KTb/QTb of
            # slot 2g+2/2g+3 only at its end, V/T of those slots only by
            # group g+1's triangle/const pairs).
            emit_K0_half(0)
            emit_Q(0)
            PENDINGS = [
                [lambda: emit_K0_half(1),
                 lambda: emit_V(0), lambda: emit_T(0), lambda: emit_K(1),
                 lambda: emit_V(1), lambda: emit_T(1), lambda: emit_K(2),
                 lambda: emit_Q(2), lambda: emit_K(3)],
                [lambda: emit_V(2), lambda: emit_T(2), lambda: emit_V(3),
                 lambda: emit_T(3), lambda: emit_K(4), lambda: emit_Q(4),
                 lambda: emit_K(5)],
                [lambda: emit_V(4), lambda: emit_T(4), lambda: emit_V(5),
                 lambda: emit_T(5), lambda: emit_K(6), lambda: emit_Q(6),
                 lambda: emit_K(7)],
                [lambda: emit_V(6), lambda: emit_T(6), lambda: emit_V(7),
                 lambda: emit_T(7)],
            ]

            # ---------- attention (software-pipelined) ----------
            # PV/sums/acc for pair p are emitted while pair p+1's S runs, and
            # each group's sums-merge + epilogue is emitted after the next
            # group's first S/exp, so the PE queue never blocks the ACT queue.
            prev_fin = None
            for g in range(NG):
                npair = 4 * (g + 1)
                pending = PENDINGS[g]
                per = (len(pending) + npair - 1) // npair if pending else 0

                otps = atp.tile([128, QG], F32, tag="otps")
                smps = atp.tile([128, QG], F32, tag="smps")
                acc = workp.tile([128, QG], BF16, tag="acc")
                qgb = QTb[:, g * QG:(g + 1) * QG]
                first_bf = npair - 4  # first bf16 (triangle) pair
                # first fp8 pair: p=0 for groups>=1 (plain), else the first
                # const pair (g0 has tri pairs at p0/p1)
                first_f8 = 0 if g > 0 else npair - 2

                def make_consume(p, ptx, fp8_pair, otps=otps, smps=smps,
                                 acc=acc, npair=npair, first_bf=first_bf,
                                 first_f8=first_f8):
                    def go():
                        if fp8_pair:
                            rhs8 = ptx.rearrange("p (j q) -> p j q", j=2)
                            nc.tensor.matmul(
                                otps,
                                lhsT=VV8r[:, 2 * p:2 * p + 2],
                                rhs=rhs8,
                                start=(p == 0), stop=(p == npair - 1),
                                perf_mode=DR,
                            )
                            nc.tensor.matmul(
                                smps,
                                lhsT=ones8,
                                rhs=rhs8,
                                start=(p == first_f8), stop=False,
                                perf_mode=DR,
                                skip_group_check=True,
                            )
                        else:
                            for hh in range(2):
                                c = 2 * p + hh
                                nc.tensor.matmul(
                                    otps,
                                    lhsT=VVr[:, c],
                                    rhs=ptx[:, hh * QG:(hh + 1) * QG],
                                    start=(p == 0 and hh == 0),
                                    stop=(p == npair - 1 and hh == 1),
                                )
                            if p == first_bf:
                                nc.vector.tensor_copy(acc, ptx[:, 0:QG])
                            else:
                                nc.vector.tensor_tensor(
                                    acc, acc, ptx[:, 0:QG], op=AluOpType.add
                                )
                            nc.vector.tensor_tensor(
                                acc, acc, ptx[:, QG:2 * QG], op=AluOpType.add
                            )
                    return go

                consume = None
                for p in range(npair):
                    kind = ("plain" if p < npair - 4
                            else ("tri" if p < npair - 2 else "const"))
                    fp8_pair = ATT8 and (
                        kind == "const" or (g > 0 and kind == "plain"))
                    sps = atp.tile([128, 2 * QG], F32, tag="sps", bufs=3)
                    for hh in range(2):
                        c = 2 * p + hh
                        nc.tensor.matmul(
                            sps[:, hh * QG:(hh + 1) * QG],
                            lhsT=KTb[:, c * 128:(c + 1) * 128],
                            rhs=qgb,
                            start=True, stop=True,
                        )
                    # exp bias: -2 everywhere (fp8 range), -32/-2 const slots
                    col = 1 if kind == "const" else 0
                    bb = biasT[:, col:col + 1]
                    if fp8_pair:
                        ptx = workp.tile([128, 2 * QG], FP8, tag="pt8",
                                         bufs=3, name="pt8")
                    else:
                        ptx = workp.tile([128, 2 * QG], BF16, tag="pt",
                                         bufs=3, name="pt")
                    nc.scalar.activation(
                        ptx, sps, mybir.ActivationFunctionType.Exp,
                        scale=SCALE, bias=bb,
                    )
                    if kind == "tri":
                        m0 = 2 * (p - (npair - 4))
                        nc.vector.tensor_tensor(
                            ptx, ptx, masks[:, m0 * QG:(m0 + 2) * QG],
                            op=AluOpType.mult,
                        )
                    if p == 0 and prev_fin is not None:
                        prev_fin()
                        prev_fin = None
                    if consume is not None:
                        consume()
                    consume = make_consume(p, ptx, fp8_pair)
                    for _ in range(per):
                        if pending:
                            pending.pop(0)()
                consume()
                while pending:
                    pending.pop(0)()

                def fin(g=g, otps=otps, smps=smps, acc=acc):
                    # merge bf16 sums into smps and finish the group
                    nc.tensor.matmul(
                        smps, lhsT=ones, rhs=acc,
                        start=(not ATT8), stop=True,
                        skip_group_check=True,
                    )
                    rb = workp.tile([128, QG], F32, tag="rb", name="rb")
                    nc.vector.reciprocal_approx_fast(rb, smps)
                    ot = workp.tile([128, QG], F32, tag="ot", name="ot")
                    nc.vector.tensor_tensor(ot, otps, rb, op=AluOpType.mult)
                    nc.sync.dma_start(outT[:, g * QG:(g + 1) * QG], ot)
                prev_fin = fin
            prev_fin()

    if not nc.is_finalized():
        nc.finalize()
    return nc


_NC_CACHE = None


def _get_program():
    global _NC_CACHE
    if _NC_CACHE is None:
        _NC_CACHE = _build_program()
    return _NC_CACHE


def _tri_masks() -> np.ndarray:
    """[4,128,QG] multiplicative triangle masks, lane-independent:
    slot m keeps kv where 128*m + kv <= q."""
    out = np.zeros((4, 128, QG), NPBF16)
    kv = np.arange(128)[:, None]
    q = np.arange(QG)[None, :]
    for m in range(4):
        out[m] = (128 * m + kv <= q).astype(NPBF16)
    return out


def _dr_pack(w: np.ndarray) -> np.ndarray:
    """[C, N] fp32 -> [128, C//256, 2, N] fp8 DoubleRow layout."""
    Cx, N = w.shape
    return np.ascontiguousarray(
        w.reshape(Cx // 256, 2, 128, N).transpose(2, 0, 1, 3)
    ).astype(NPF8)


def _run(inputs: dict, trace: bool = False, trace_kwargs: dict | None = None):
    x = np.asarray(inputs["x"], np.float32)
    Wk = np.asarray(inputs["Wk"], np.float32)
    Wq = np.asarray(inputs["Wq"], np.float32)
    Wv = np.asarray(inputs["Wv"], np.float32)

    nc = _get_program()

    wk16, wq16, wv16 = (w.astype(NPBF16) for w in (Wk, Wq, Wv))
    wk8, wq8, wv8 = (_dr_pack(w) for w in (Wk, Wq, Wv))
    tri = _tri_masks()

    in_maps = []
    for b in range(B):
        xtb = np.ascontiguousarray(x[b].T)  # [C, T] f32
        xtb16 = xtb.astype(NPBF16)
        for j in range(2):
            order = [s ^ j for s in range(TGRP)]
            xbf = np.concatenate(
                [xtb16[:, order[s] * QG:(order[s] + 1) * QG]
                 for s in range(TGRP)], axis=1,
            )
            x8 = np.stack(
                [
                    _dr_pack(xtb[:, order[s] * QG:(order[s] + 1) * QG])
                    .reshape(128, 4, 2, QG)
                    for s in range(2, TGRP)
                ], axis=1,
            )  # [128, 6, 4, 2, QG]
            bia = np.zeros((128, 4), np.float32)
            bia[:, 0] = -2.0
            bia[:, 1] = -32.0 if j == 0 else -2.0
            bia[:, 2] = -2.0
            bia[:, 3] = -32.0 if j == 0 else -2.0
            in_maps.append(
                {
                    "xbf": np.ascontiguousarray(xbf),
                    "x8": np.ascontiguousarray(x8),
                    "wk": wk16, "wq": wq16, "wv": wv16,
                    "wk8": wk8, "wq8": wq8, "wv8": wv8,
                    "msk": tri,
                    "bia": bia,
                }
            )

    res = run_bass_kernel_spmd(
        nc,
        in_maps,
        core_ids=list(range(NCORES)),
        trace=trace,
        **(trace_kwargs or {}),
    )

    out = np.empty((B, T, H), np.float32)
    for core in range(NCORES):
        b, j = divmod(core, 2)
        oT = np.asarray(res.results[core]["outT"], np.float32)
        for g in range(NG):
            row = (2 * g + j) * QG
            out[b, row:row + QG, :] = oT[:, g * QG:(g + 1) * QG].T
    return out, res


def kernel(**inputs) -> np.ndarray:
    out, _ = _run(inputs, trace=False)
    return out
